# revision 41
# baseline (speedup 1.0000x reference)
"""Trainium2 Bass kernel for nn_DifferentiableStarPlanner.

Algorithm notes (validated bitwise vs the reference in numpy):

  * The reference's open/close/pool computations never feed the returned
    tensor: the output is exactly NUM_SWEEPS Jacobi sweeps of a 9-channel
    min-plus stencil  g <- min(g, min_c(shift_c(g) + cmap_c))  with
    g0 = 1e7 everywhere except the start cell.
  * Only the start bounding box inflated by NUM_SWEEPS (clipped) can change
    from 1e7: a 113x113 corner here.  Edge-replicate padding is replaced by
    1e7 guard cells (provably never the argmin), the center channel by a
    pure-copy identity channel.
  * Per sweep only cells within t steps of the start can change, so all
    per-sweep work is windowed to the active wavefront (rows and cols both
    grow by 1 per sweep).

Device mapping (one NeuronCore; all 8 cores run identical replicas).

  * State g stays row-major fp16 (scaled by 2^-10) the whole run.  The three
    row-shift variants dy in {-1,0,+1} are three regular (non-transpose)
    matmuls per sweep: stationary lhsT is a KxK cyclic 0/1 matrix (wrap rows
    land on rows that are still INF, or on the two INF guard partitions at
    K=Dr+2); the moving rhs is ONE 2-level AP covering the three overlapping
    dx col-windows, so each matmul fills a whole PSUM bank (3 channel
    regions).  fp16 ifmap streams at 1 cyc/row; PSUM accumulates in exact
    f32.
  * cmap is added for free: per bank, one identity-lhsT matmul routes the 3
    (fp16, scaled) cmap channel slabs into the bank ahead of the shifts
    (center slab is zeros).  These preloads for sweep t+1 fill the other
    PSUM bank set during sweep t, hidden under the reduce.
  * All constants (initial state, identity, cyclic matrices, the 9 packed
    cmap slabs) are precomputed on the host in numpy and DMAed in as one
    fp16 tensor: on-device setup is just the DMAs.
  * The 9-way min is one DVE tensor_reduce over a 3x3x{window} PSUM access
    pattern, writing the window back into g in place (fp16 round once per
    sweep; end-to-end max rel err vs f32 reference ~3.4e-3, gate is 2e-2).
  * Filler matmuls keep the PE from idling between sweeps: the PE p-state
    drops to 1.2 GHz after idle gaps (measured: LDWEIGHTS 80ns -> 159ns).
"""
import sys
import os
import numpy as np

for _p in ("/opt/trn_rl_repo", "/root/.axon_site/_ro/trn_rl_repo"):
    if os.path.isdir(_p) and _p not in sys.path:
        sys.path.insert(0, _p)

import concourse.bass as bass
import concourse.bacc as bacc
import concourse.mybir as mybir
from concourse import tile
from concourse.bass_utils import run_bass_kernel_spmd

F32 = mybir.dt.float32
F16 = mybir.dt.float16
ALU = mybir.AluOpType
AXL = mybir.AxisListType

INF = np.float32(1.0e7)
OB_COST = 10000.0
EPS = 1e-12
NUM_SWEEPS = 80
N_CORES = 8
SCALE = float(np.float32(2.0 ** -10))

SS = 116       # psum region stride within a bank (f32 elements)
PS_BANK = 512  # psum bank stride (f32 elements)
N_FILLERS = int(os.environ.get("K_FILL", "0"))


K_STEPS = ((40, 48, 56, 64, 72, 80, 88, 96, 104, 112)
           if os.environ.get("K_FINE", "") else (48, 64, 80, 96, 112))


def _window(t, D, slo, shi):
    return max(0, slo - t), min(D - 1, shi + t)


def _K_of(t, Dr, seed_rlo, seed_rhi):
    # stationary-weight K: quantized so the PE weight matrices only change
    # twice over the whole run.
    _, rhi = _window(t, Dr, seed_rlo, seed_rhi)
    need = rhi + 3
    cap = Dr + 2
    for step in K_STEPS:
        if need <= step <= cap:
            return step
    return cap


def _cyc_Ks(Dr, seed_rlo, seed_rhi, num_sweeps):
    ks = {_K_of(t, Dr, seed_rlo, seed_rhi) for t in range(1, num_sweeps + 1)}
    ks.add(Dr + 2)
    return sorted(ks)


def build_program(Dr, Dc, seed_rlo, seed_rhi, seed_clo, seed_chi, r0, c0,
                  H, W, num_sweeps):
    """Domain = grid rows r0..r0+Dr-1, cols c0..c0+Dc-1; seed_* in domain coords."""
    Sr, Sc = Dr + 2, Dc + 2
    assert Dr + 2 <= 128 and Dc + 2 <= 128
    Ks = _cyc_Ks(Dr, seed_rlo, seed_rhi, num_sweeps)

    nc = bacc.Bacc("TRN2", target_bir_lowering=False, debug=False)

    # ---- DRAM I/O: one packed fp16 tensor of precomputed constants ----
    # order: everything the first sweeps need, then the later cyclic
    # matrices (loaded by a second DMA off the critical path)
    seg = [("ginit", Sc), ("ident", Sr),
           (f"cycm1_{Ks[0]}", Ks[0]), (f"cycp1_{Ks[0]}", Ks[0]),
           ("cmapP", 9 * Dc)]
    for K in Ks[1:]:
        seg.append((f"cycm1_{K}", K))
        seg.append((f"cycp1_{K}", K))
    offs, TOT = {}, 0
    for nm, wd in seg:
        offs[nm] = TOT
        TOT += wd
    d_pack = nc.dram_tensor("packed16", [Sr, TOT], F16, kind="ExternalInput")
    d_out = nc.dram_tensor("out", [H, W], F32, kind="ExternalOutput")

    with tile.TileContext(nc) as tc:
        from contextlib import ExitStack
        with ExitStack() as ctx:
            sb = ctx.enter_context(tc.tile_pool(name="sb", bufs=1))
            ps = ctx.enter_context(tc.tile_pool(name="ps", bufs=1, space="PSUM"))

            pk = d_pack.ap()

            def seg_ap(nm, wd, rows):
                return pk[0:rows, offs[nm]:offs[nm] + wd]

            # state: partition p = row p (rows Dr..Dr+1 are INF guards);
            # free f = col f-1 with INF guard cells at f=0 and f=Dc+1
            g_rm = sb.tile([Sr, Sc], F16)
            # all read-only constants live in ONE tile: a single DMA fills
            # them (each dma_start costs ~0.6-0.9us of issue time on the
            # sync queue)
            t_const = sb.tile([Sr, TOT - Sc], F16)

            def cslice(nm, p1, f0, f1):
                o = offs[nm] - Sc
                return t_const[0:p1, o + f0:o + f1]

            bg = sb.tile([128, W], F32)
            t_fin = sb.tile([Dr, Dc], F32)

            psum_sets = [ps.tile([128, 3 * PS_BANK], F32, name="psumA"),
                         ps.tile([128, 3 * PS_BANK], F32, name="psumB")]
            psD = ps.tile([128, PS_BANK], F32, name="psD")

            def shiftM(K, d, PF):
                if d == 0:
                    return cslice("ident", K, 0, PF)
                return cslice(f"cyc{'m1' if d == -1 else 'p1'}_{K}", K, 0, PF)

            # ---- load all constants: state DMA + two constants DMAs ----
            # (the second carries only the late-sweep cyclic matrices, so the
            # first sweeps start as soon as the critical chunk lands)
            # constants first: the first preload waits on them, while the
            # state is only needed one matmul-group later
            X = offs["cmapP"] + 9 * Dc
            nc.sync.dma_start(t_const[0:Sr, 0:X - Sc], pk[0:Sr, Sc:X])
            nc.sync.dma_start(g_rm[:], seg_ap("ginit", Sc, Sr))
            nc.sync.dma_start(t_const[0:Sr, X - Sc:TOT - Sc], pk[0:Sr, X:TOT])
            v = nc.vector
            v.memset(bg[:], INF)

            # ---- background writes (1e7 outside the domain) ----
            out_ap = d_out.ap()
            bg_rows = []
            if r0 > 0:
                bg_rows.append((0, r0))
            if r0 + Dr < H:
                bg_rows.append((r0 + Dr, H))
            for lo_, hi_ in bg_rows:
                r = lo_
                while r < hi_:
                    n = min(128, hi_ - r)
                    nc.sync.dma_start(out_ap[r:r + n, :], bg[0:n, :])
                    r += n
            if c0 > 0:
                nc.sync.dma_start(out_ap[r0:r0 + Dr, 0:c0], bg[0:Dr, 0:c0])
            if c0 + Dc < W:
                nc.sync.dma_start(out_ap[r0:r0 + Dr, c0 + Dc:W],
                                  bg[0:Dr, 0:W - c0 - Dc])

            # ---- helpers ----
            def ap3(tile_ap, col_off, dims):
                base = tile_ap
                pap = list(base.ap)
                return bass.AP(base.tensor, base.offset + col_off,
                               [list(pap[0])] + [list(d) for d in dims])

            def win(t):
                rlo, rhi = _window(t, Dr, seed_rlo, seed_rhi)
                clo, chi = _window(t, Dc, seed_clo, seed_chi)
                return rlo, rhi, clo, chi

            def preload(set_idx, t2):
                # route the 9 cmap channel slabs (center = zeros) for sweep t2
                # into their PSUM regions via identity-lhsT matmuls, merging
                # banks while a matmul's total PSUM write fits 2KB.
                K = _K_of(t2, Dr, seed_rlo, seed_rhi)
                PF = min(Dr, K)
                _, _, clo, chi = win(t2)
                wc = chi - clo + 1
                # NOTE: merging banks into one matmul (4-dim out AP) compiles
                # but writes wrong addresses on HW — keep strictly per-bank.
                nb = 1
                b = 0
                while b < 3:
                    n = min(nb, 3 - b)
                    odims = ([[SS, 3], [1, wc]] if n == 1 else
                             [[PS_BANK, n], [SS, 3], [1, wc]])
                    out = ap3(psum_sets[set_idx][0:PF, 0:3 * PS_BANK],
                              b * PS_BANK + clo, odims)
                    rhs = ap3(cslice("cmapP", K, 0, 9 * Dc), 3 * b * Dc + clo,
                              [[Dc, 3 * n], [1, wc]])
                    nc.tensor.matmul(out, lhsT=cslice("ident", K, 0, PF), rhs=rhs,
                                     is_transpose=False, start=True, stop=False,
                                     skip_group_check=True)
                    b += n

            KF = Ks[0]

            def emit_fillers(n):
                for _ in range(n):
                    nc.tensor.matmul(psD[0:KF, 0:KF],
                                     lhsT=cslice("ident", KF, 0, KF),
                                     rhs=cslice("ident", KF, 0, KF),
                                     is_transpose=False,
                                     start=True, stop=True,
                                     skip_group_check=True)

            preload(0, 1)

            def shifts(t, cur, K, PF, a, b):
                # shifts for out cols [a, b]: one matmul per dy covers all 3
                # dx via a 2-level rhs AP over the overlapping windows
                w = b - a + 1
                for dy in (-1, 1, 0):
                    off = (dy + 1) * PS_BANK + a
                    out = ap3(cur[0:PF, 0:3 * PS_BANK], off, [[SS, 3], [1, w]])
                    rhs = ap3(g_rm[0:K, 0:Dc + 2], a, [[1, 3], [1, w]])
                    nc.tensor.matmul(out, lhsT=shiftM(K, dy, PF), rhs=rhs,
                                     is_transpose=False, start=False, stop=True,
                                     skip_group_check=True)

            def reduce(cur, rhi, a, b):
                # PSUM accesses must start at partition 0; rows above the
                # window reduce to their own INF (center channel), a no-op.
                w = b - a + 1
                base = cur[0:rhi + 1, 0:3 * PS_BANK]
                in_ap = ap3(base, a, [[1, w], [PS_BANK, 3], [SS, 3]])
                v.tensor_reduce(g_rm[0:rhi + 1, 1 + a:1 + a + w], in_ap,
                                axis=AXL.XY, op=ALU.min)

            # ---- sweeps ----
            # Split pipelining: the upper (wavefront-edge) half of the window
            # is shifted and reduced first, so the next sweep's upper shifts
            # overlap this sweep's interior reduce.  cmid must advance by at
            # least 1 per sweep so the upper half of sweep t+1 never reads
            # what the interior reduce of sweep t writes.
            # Split pipelining measured neutral-to-negative: the Tile dep
            # tracker serializes the next sweep's upper shifts behind the
            # interior reduce anyway, and the extra weight loads cost PE
            # time.  Keep it available but off.
            USE_SPLIT = os.environ.get("K_SPLIT", "") != ""
            cmid_prev = None
            for t in range(1, num_sweeps + 1):
                cur = psum_sets[(t - 1) % 2]
                rlo, rhi, clo, chi = win(t)
                K = _K_of(t, Dr, seed_rlo, seed_rhi)
                PF = min(Dr, K)
                wc = chi - clo + 1

                cmid = None
                if USE_SPLIT and wc >= 40 and t < num_sweeps:
                    cmid = (clo + chi + 1) // 2
                    if cmid_prev is not None:
                        # +2 keeps a full untouched position (one 4-byte SBUF
                        # word) between the interior reduce's writes and the
                        # next sweep's upper-shift reads, so range tracking at
                        # word granularity sees them as disjoint
                        cmid = max(cmid, cmid_prev + 2)
                    if cmid > chi - 12:
                        cmid = None   # upper half too thin: re-center via one
                                      # single-phase sweep

                if cmid is None:
                    shifts(t, cur, K, PF, clo, chi)
                    if t < num_sweeps:
                        preload(t % 2, t + 1)
                    reduce(cur, rhi, clo, chi)
                else:
                    shifts(t, cur, K, PF, cmid, chi)   # upper first
                    shifts(t, cur, K, PF, clo, cmid - 1)
                    preload(t % 2, t + 1)
                    reduce(cur, rhi, cmid, chi)        # unblocks sweep t+1 upper
                    reduce(cur, rhi, clo, cmid - 1)
                cmid_prev = cmid

            # ---- write out (state is row-major already) ----
            v.tensor_scalar_mul(t_fin[:], g_rm[0:Dr, 1:1 + Dc], 1.0 / SCALE)
            nc.sync.dma_start(out_ap[r0:r0 + Dr, c0:c0 + Dc], t_fin[:])

    nc.compile()
    return nc, ["packed16"]


def _cmap_channels(obs, coords):
    """Full-grid 9-channel step costs, reference channel order c = x*3+y."""
    h, w = obs.shape
    yc, xc = coords[0, 0], coords[0, 1]
    Lsq = (xc - np.concatenate([xc[:, :1], xc[:, :-1]], 1)) ** 2
    Rsq = (xc - np.concatenate([xc[:, 1:], xc[:, -1:]], 1)) ** 2
    Usq = (yc - np.concatenate([yc[1:, :], yc[-1:, :]], 0)) ** 2
    Dsq = (yc - np.concatenate([yc[:1, :], yc[:-1, :]], 0)) ** 2
    op = np.pad(obs, 1, mode='edge')
    nb = lambda dy, dx: op[1 + dy:1 + dy + h, 1 + dx:1 + dx + w]
    ctr = nb(0, 0)
    oc = np.float32(OB_COST)
    chans = [
        np.sqrt(Lsq + Usq + EPS) + oc * np.maximum(nb(-1, -1), ctr),
        np.sqrt(Lsq + EPS) + oc * np.maximum(nb(-1, 0), ctr),
        np.sqrt(Lsq + Dsq + EPS) + oc * np.maximum(nb(1, -1), ctr),
        np.sqrt(Usq + EPS) + oc * np.maximum(nb(-1, 0), ctr),
        np.zeros_like(ctr),
        np.sqrt(Dsq + EPS) + oc * np.maximum(nb(1, 0), ctr),
        np.sqrt(Rsq + Usq + EPS) + oc * np.maximum(nb(-1, 1), ctr),
        np.sqrt(Rsq + EPS) + oc * np.maximum(nb(0, 1), ctr),
        np.sqrt(Rsq + Dsq + EPS) + oc * np.maximum(nb(1, 1), ctr),
    ]
    return [c.astype(np.float32) for c in chans]


def prep_inputs(obstacles, coords, start_map, num_sweeps=NUM_SWEEPS):
    """Host-side precompute of all device constants (fp16). Returns (in_map, geom)."""
    obs = np.asarray(obstacles, np.float32)[0, 0]
    co = np.asarray(coords, np.float32)
    s = np.asarray(start_map, np.float32)[0, 0]
    H, W = obs.shape

    ys, xs = np.nonzero(s > 0)
    assert len(ys) >= 1, "empty start_map"
    r0 = max(0, int(ys.min()) - num_sweeps)
    r1 = min(H - 1, int(ys.max()) + num_sweeps)
    c0 = max(0, int(xs.min()) - num_sweeps)
    c1 = min(W - 1, int(xs.max()) + num_sweeps)
    Dr, Dc = r1 - r0 + 1, c1 - c0 + 1
    Sr, Sc = Dr + 2, Dc + 2
    seeds = (int(ys.min()) - r0, int(ys.max()) - r0,
             int(xs.min()) - c0, int(xs.max()) - c0)
    Ks = _cyc_Ks(Dr, seeds[0], seeds[1], num_sweeps)
    S16 = np.float16
    SC = np.float32(SCALE)

    # initial state with guards
    gi = np.full((Sr, Sc), INF * SC, np.float32)
    g0 = np.clip(INF * (np.float32(1.0) - s[r0:r1 + 1, c0:c1 + 1]), 0.0, INF)
    gi[0:Dr, 1:1 + Dc] = g0 * SC

    def cyc(n, d):
        P = np.zeros((n, n), np.float32)
        P[(np.arange(n) + d) % n, np.arange(n)] = 1.0
        return P

    # 9 cmap slabs in (dy,dx)-major order, scaled; center slab = zeros
    chans = _cmap_channels(obs, co)
    cmapP = np.zeros((Sr, 9 * Dc), np.float32)
    for dy in (-1, 0, 1):
        for dx in (-1, 0, 1):
            ci = (dy + 1) * 3 + (dx + 1)
            if dy == 0 and dx == 0:
                continue
            ref_c = (dx + 1) * 3 + (dy + 1)
            cmapP[0:Dr, ci * Dc:(ci + 1) * Dc] = \
                chans[ref_c][r0:r1 + 1, c0:c1 + 1] * SC

    parts = [gi, np.eye(Sr, dtype=np.float32),
             cyc(Ks[0], -1), cyc(Ks[0], 1), cmapP]
    widths = [Sc, Sr, Ks[0], Ks[0], 9 * Dc]
    for K in Ks[1:]:
        parts += [cyc(K, -1), cyc(K, 1)]
        widths += [K, K]

    TOT = sum(widths)
    packed = np.zeros((Sr, TOT), S16)
    o = 0
    for a, wd in zip(parts, widths):
        packed[0:a.shape[0], o:o + wd] = a.astype(S16)
        o += wd
    in_map = {"packed16": np.ascontiguousarray(packed)}

    geom = dict(Dr=Dr, Dc=Dc, r0=r0, c0=c0, H=H, W=W,
                seed_rlo=seeds[0], seed_rhi=seeds[1],
                seed_clo=seeds[2], seed_chi=seeds[3])
    return in_map, geom


def kernel(obstacles, coords, start_map, goal_map):
    in_map, gm = prep_inputs(obstacles, coords, start_map)
    nc, _ = build_program(gm["Dr"], gm["Dc"], gm["seed_rlo"], gm["seed_rhi"],
                          gm["seed_clo"], gm["seed_chi"], gm["r0"], gm["c0"],
                          gm["H"], gm["W"], NUM_SWEEPS)
    in_maps = [in_map for _ in range(N_CORES)]
    res = run_bass_kernel_spmd(nc, in_maps, core_ids=list(range(N_CORES)))
    out = res.results[0]["out"]
    return np.ascontiguousarray(out[None, None]).astype(np.float32)
